# revision 15
# baseline (speedup 1.0000x reference)
"""Trainium2 Bass kernel for LocalAttention: sliding-window attention gate +
per-position linear + tanh + global maxpool.

out[b,c] = tanh(max_l( sigmoid(conv1d(x, W_att) + b_att)[l] * (W_cnn @ x[b].T)[c,l] ) + b_cnn[c])

Sharding: data-parallel over batch B=64 across 8 cores (8 batches/core).

Key design points (v2):
- x is cast to bf16 AND pre-transposed to [EC, 128, L] on the host, so the
  device does no cast and no on-chip transpose.
- W_cnn (200 rows) and W_att (5 rows) are packed into one 229-row augmented
  weight so the sliding-window score channels ride along in the main matmul.
- The 5 shifted score rows are realized with a single "diagonal" DMA whose
  partition stride also advances one element (stride = pitch+1).
- Gating multiply + max-reduction fuse into one DVE tensor_tensor_reduce.
- The PE instruction stream is software-pipelined: [ci1(b+1)] [ones(b)]
  [ci0(b)] so matmul dispatch never waits on the score chain.
"""

import functools
import sys

import ml_dtypes
import numpy as np

sys.path.insert(0, "/opt/trn_rl_repo")

import concourse.bacc as bacc
import concourse.tile as tile
from concourse import mybir
from concourse.bass_utils import run_bass_kernel_spmd

B, L, E, WIN, C = 64, 1024, 512, 5, 200
NCORES = 8
BS = B // NCORES  # batches per core
P = 128
EC = E // P       # 4 e-chunks (contraction over E in 128-slices)
NLT = 2           # L-tiles for matmul free dim
LTW = L // NLT    # 512
# augmented output channels: 200 cnn + zero-pad to 32-align + 5 att rows.
# u rows must start at a 32-aligned partition for compute-engine PSUM reads.
UOFF = 96         # local partition offset of the W_att rows inside c-chunk 1
CAUG = P + UOFF + WIN  # 229
# c-chunks of the augmented output: (start, width, valid_out_width)
CCH = [(0, P, P), (P, UOFF + WIN, C - P)]
NEG = -3.0e38

FP32 = mybir.dt.float32
BF16 = mybir.dt.bfloat16
AF = mybir.ActivationFunctionType
ALU = mybir.AluOpType


def _body(nc, tc, x_d, w_d, ones_d, batt_d, bcnn_d, out_d):
    with (
        tc.tile_pool(name="const", bufs=1) as cpool,
        tc.tile_pool(name="xt", bufs=4) as xtpool,
        tc.tile_pool(name="u", bufs=2) as upool,
        tc.tile_pool(name="ua", bufs=2) as uapool,
        tc.tile_pool(name="s", bufs=2) as spool,
        tc.tile_pool(name="g", bufs=2) as gpool,
        tc.tile_pool(name="oacc", bufs=1) as opool,
        tc.tile_pool(name="pv1", bufs=2, space="PSUM") as pv1pool,
        tc.tile_pool(name="pv0", bufs=1, space="PSUM") as pv0pool,
        tc.tile_pool(name="ps", bufs=2, space="PSUM") as pspool,
    ):
        # ---- weights first, then x0/x1, then the cold constants, so the
        # first matmul's inputs land as early as possible on the shared DMA path
        w_sb = cpool.tile([P, EC, CAUG], BF16, tag="w")
        nc.sync.dma_start(out=w_sb[:], in_=w_d.rearrange("ec p c -> p ec c"))

        oacc = [
            opool.tile([P, BS], FP32, tag=f"oacc{ci}", name=f"oacc{ci}")
            for ci in range(2)
        ]

        # ---- per-batch state kept across pipeline stages ----
        xt = [None] * BS     # SBUF x tiles [P, EC, L]
        pv1 = [None] * BS    # psum tile for c-chunk 1 (with u rows), [P, L]
        pv0 = [None] * BS    # psum tile for c-chunk 0, [P, L]
        uali = [None] * BS   # aligned u rows [WIN, L]
        ssb = [None] * BS    # sigmoid scores [P, L] fp32

        def load_x(b):
            t = xtpool.tile([P, EC, L], BF16, tag="xt", name=f"xt{b}")
            # two DMAs per batch so the shared DMA engine slot frees often
            nc.sync.dma_start(
                out=t[:, 0:2, :], in_=x_d[b, 0:2].rearrange("ec p l -> p ec l")
            )
            nc.sync.dma_start(
                out=t[:, 2:4, :], in_=x_d[b, 2:4].rearrange("ec p l -> p ec l")
            )
            xt[b] = t

        def mm_chunk(b, ci):
            c0, cw, _ = CCH[ci]
            pool = pv1pool if ci == 1 else pv0pool
            pv = pool.tile([P, L], FP32, tag=f"pv{ci}", name=f"pv{ci}_{b}")
            for lt in range(NLT):
                for ec in range(EC):
                    nc.tensor.matmul(
                        pv[:cw, lt * LTW : (lt + 1) * LTW],
                        lhsT=w_sb[:, ec, c0 : c0 + cw],
                        rhs=xt[b][:, ec, lt * LTW : (lt + 1) * LTW],
                        start=(ec == 0),
                        stop=(ec == EC - 1),
                    )
            if ci == 1:
                pv1[b] = pv
            else:
                pv0[b] = pv

        def score_prep(b):
            # u rows -> usb (zero-padded by 2 on both ends), then one diagonal
            # DMA builds all 5 shifted rows at once.
            usb = upool.tile([WIN, L + 4], BF16, tag="usb", name=f"usb{b}")
            nc.gpsimd.memset(usb[:, 0:2], 0.0)
            nc.gpsimd.memset(usb[:, L + 2 : L + 4], 0.0)
            for lt in range(NLT):
                nc.scalar.copy(
                    out=usb[:, 2 + lt * LTW : 2 + (lt + 1) * LTW],
                    in_=pv1[b][UOFF : UOFF + WIN, lt * LTW : (lt + 1) * LTW],
                )
            ua = uapool.tile([WIN, L], BF16, tag="uali", name=f"uali{b}")
            src = usb[:, 0:L].copy()
            d = src.ap
            d[0] = (d[0][0] + 1, WIN)  # diagonal: row w starts w elements later
            nc.scalar.dma_start(out=ua[:], in_=src)
            uali[b] = ua

        def score_mm(b):
            # broadcast-sum the 5 aligned rows to all partitions, sigmoid
            s = spool.tile([P, L], FP32, tag="ssb", name=f"ssb{b}")
            for lt in range(NLT):
                ps = pspool.tile([P, LTW], FP32, tag="ps", name=f"ps{b}_{lt}")
                nc.tensor.matmul(
                    ps[:],
                    lhsT=ones_sb[:],
                    rhs=uali[b][:, lt * LTW : (lt + 1) * LTW],
                    start=True,
                    stop=True,
                )
                nc.scalar.activation(
                    out=s[:, lt * LTW : (lt + 1) * LTW],
                    in_=ps[:],
                    func=AF.Sigmoid,
                    bias=batt_sb[:],
                )
            ssb[b] = s

        def gate_reduce(b, ci):
            # gate-multiply (one 1024-wide DVE op) then max over l on DVE
            _, _, cwo = CCH[ci]
            pv = pv1[b] if ci == 1 else pv0[b]
            g = gpool.tile([P, L], FP32, tag="gout", name=f"g{b}_{ci}")
            nc.vector.tensor_mul(
                out=g[:cwo, :], in0=pv[:cwo, :], in1=ssb[b][:cwo, :]
            )
            nc.vector.reduce_max(
                oacc[ci][:cwo, b : b + 1],
                g[:cwo, :],
                axis=mybir.AxisListType.X,
            )
            if ci == 1:
                pv1[b] = None
            else:
                pv0[b] = None

        # ---- software-pipelined main loop ----
        load_x(0)
        if BS > 1:
            load_x(1)
        # cold constants go behind x0/x1 on the DMA path
        ones_sb = cpool.tile([WIN, P], BF16, tag="ones")
        nc.sync.dma_start(out=ones_sb[:], in_=ones_d)
        batt_sb = cpool.tile([P, 1], FP32, tag="batt")
        nc.sync.dma_start(out=batt_sb[:], in_=batt_d)
        bcnn_sb = []
        for ci, (c0, cw, cwo) in enumerate(CCH):
            t = cpool.tile([cwo, 1], FP32, tag=f"bcnn{ci}")
            nc.sync.dma_start(out=t[:], in_=bcnn_d[c0 : c0 + cwo, :])
            bcnn_sb.append(t)

        for b in range(BS):
            if b + 2 < BS:
                load_x(b + 2)
            mm_chunk(b, 1)       # PE: c-chunk 1 (contains u rows)
            score_prep(b)        # Act: u copy + diagonal shift DMA
            if b >= 1:
                score_mm(b - 1)      # PE: ones matmul; Act: sigmoid
                gate_reduce(b - 1, 1)  # DVE: gate+max, frees pv1(b-1)
                mm_chunk(b - 1, 0)   # PE: c-chunk 0
                gate_reduce(b - 1, 0)  # DVE: gate+max, frees pv0(b-1)
        b = BS - 1
        score_mm(b)
        gate_reduce(b, 1)
        mm_chunk(b, 0)
        gate_reduce(b, 0)

        # ---- tanh(max + b_cnn) and store ----
        for ci, (c0, cw, cwo) in enumerate(CCH):
            of = gpool.tile([P, BS], FP32, tag=f"of{ci}")
            nc.scalar.activation(
                out=of[:cwo, :], in_=oacc[ci][:cwo, :], func=AF.Tanh,
                bias=bcnn_sb[ci][:],
            )
            nc.sync.dma_start(out=out_d[c0 : c0 + cwo, :], in_=of[:cwo, :])


@functools.lru_cache(maxsize=1)
def _build():
    nc = bacc.Bacc(
        "TRN2",
        target_bir_lowering=False,
        debug=False,
        enable_asserts=False,
        num_devices=NCORES,
    )
    x_d = nc.dram_tensor("xT", [BS, EC, P, L], BF16, kind="ExternalInput").ap()
    w_d = nc.dram_tensor("waugT", [EC, P, CAUG], BF16, kind="ExternalInput").ap()
    ones_d = nc.dram_tensor("ones5", [WIN, P], BF16, kind="ExternalInput").ap()
    batt_d = nc.dram_tensor("b_att_b", [P, 1], FP32, kind="ExternalInput").ap()
    bcnn_d = nc.dram_tensor("b_cnn_c", [C, 1], FP32, kind="ExternalInput").ap()
    out_d = nc.dram_tensor("out", [C, BS], FP32, kind="ExternalOutput").ap()
    with tile.TileContext(nc) as tc:
        _body(nc, tc, x_d, w_d, ones_d, batt_d, bcnn_d, out_d)
    nc.compile()
    return nc


def _prep_in_maps(x, W_att, b_att, W_cnn, b_cnn):
    pad = np.zeros((CAUG - C - WIN, E), dtype=np.float32)
    waug = np.concatenate([W_cnn, pad, W_att], axis=0)     # [229, 512]
    waugT = np.ascontiguousarray(waug.T)                   # [512, 229]
    waugT = waugT.reshape(EC, P, CAUG).astype(ml_dtypes.bfloat16)
    ones5 = np.ones((WIN, P), dtype=ml_dtypes.bfloat16)
    batt = np.full((P, 1), np.float32(b_att[0]), dtype=np.float32)
    bcnn = np.asarray(b_cnn, dtype=np.float32).reshape(C, 1)
    # host-side cast + transpose: [B, L, E] -> bf16 [B, EC, P, L]
    xT = np.ascontiguousarray(
        np.asarray(x).astype(ml_dtypes.bfloat16).transpose(0, 2, 1)
    ).reshape(B, EC, P, L)
    in_maps = []
    for c in range(NCORES):
        in_maps.append(
            {
                "xT": xT[c * BS : (c + 1) * BS],
                "waugT": waugT,
                "ones5": ones5,
                "b_att_b": batt,
                "b_cnn_c": bcnn,
            }
        )
    return in_maps


def run(x, W_att, b_att, W_cnn, b_cnn, trace=False):
    nc = _build()
    in_maps = _prep_in_maps(x, W_att, b_att, W_cnn, b_cnn)
    res = run_bass_kernel_spmd(nc, in_maps, core_ids=list(range(NCORES)), trace=trace)
    outs = [r["out"] for r in res.results]  # each [C, BS]
    out = np.concatenate([o.T for o in outs], axis=0)  # [B, C]
    return out[:, :, None, None].astype(np.float32), res


def kernel(x, W_att, b_att, W_cnn, b_cnn):
    out, _ = run(x, W_att, b_att, W_cnn, b_cnn, trace=False)
    return out


# revision 16
# speedup vs baseline: 1.1216x; 1.1216x over previous
"""Trainium2 Bass kernel for LocalAttention: sliding-window attention gate +
per-position linear + tanh + global maxpool.

out[b,c] = tanh(max_l( sigmoid(conv1d(x, W_att) + b_att)[l] * (W_cnn @ x[b].T)[c,l] ) + b_cnn[c])

Sharding: data-parallel over batch B=64 across 8 cores (8 batches/core).

Key design points:
- x is cast to bf16 AND pre-transposed to [EC, 128, L] on the host, so the
  device does no cast and no on-chip transpose.
- W_cnn (200 rows) and W_att (5 rows) are packed into one 229-row augmented
  weight so the sliding-window score channels ride along in the main matmul.
- The 5 shifted score rows are realized with a single "diagonal" DMA whose
  partition stride also advances one element (stride = pitch+1); it is issued
  on the SP queue ahead of the x prefetches so it never queues behind them.
- One unified single-bank PSUM pool (8 bufs) lets c-chunk-1 tiles survive
  ~2 batches, giving the score chain (u-copy -> diag DMA -> ones matmul ->
  sigmoid) enough slack to stay off the critical path.
- Activation tables (Sigmoid/Tanh) are preloaded in the prologue.
"""

import functools
import sys

import ml_dtypes
import numpy as np

sys.path.insert(0, "/opt/trn_rl_repo")

import concourse.bacc as bacc
import concourse.tile as tile
from concourse import mybir
from concourse.bass_utils import run_bass_kernel_spmd

B, L, E, WIN, C = 64, 1024, 512, 5, 200
NCORES = 8
BS = B // NCORES  # batches per core
P = 128
EC = E // P       # 4 e-chunks (contraction over E in 128-slices)
NLT = 2           # L-tiles for matmul free dim
LTW = L // NLT    # 512
# augmented output channels: 200 cnn + zero-pad to 32-align + 5 att rows.
# u rows must start at a 32-aligned partition for compute-engine PSUM reads.
UOFF = 96         # local partition offset of the W_att rows inside c-chunk 1
CAUG = P + UOFF + WIN  # 229
# c-chunks of the augmented output: (start, width, valid_out_width)
CCH = [(0, P, P), (P, UOFF + WIN, C - P)]

FP32 = mybir.dt.float32
BF16 = mybir.dt.bfloat16
AF = mybir.ActivationFunctionType
ALU = mybir.AluOpType


def _body(nc, tc, x_d, w_d, ones_d, batt_d, bcnn_d, out_d):
    with (
        tc.tile_pool(name="const", bufs=1) as cpool,
        tc.tile_pool(name="xt", bufs=4) as xtpool,
        tc.tile_pool(name="u", bufs=2) as upool,
        tc.tile_pool(name="ua", bufs=2) as uapool,
        tc.tile_pool(name="s", bufs=2) as spool,
        tc.tile_pool(name="g", bufs=2) as gpool,
        tc.tile_pool(name="oacc", bufs=1) as opool,
        tc.tile_pool(name="pp", bufs=8, space="PSUM") as ppool,
    ):
        # ---- weights first so the first matmul's inputs land ASAP ----
        w_sb = cpool.tile([P, EC, CAUG], BF16, tag="w")
        nc.sync.dma_start(out=w_sb[:], in_=w_d.rearrange("ec p c -> p ec c"))

        oacc = [
            opool.tile([P, BS], FP32, tag=f"oacc{ci}", name=f"oacc{ci}")
            for ci in range(2)
        ]

        # ---- per-batch state kept across pipeline stages ----
        xt = [None] * BS     # SBUF x tiles [P, EC, L]
        pv1 = [None] * BS    # psum tiles for c-chunk 1 (with u rows), per lt
        pv0 = [None] * BS    # psum tiles for c-chunk 0, per lt
        uali = [None] * BS   # aligned u rows [WIN, L]
        ssb = [None] * BS    # sigmoid scores [P, L] fp32

        def load_x(b):
            t = xtpool.tile([P, EC, L], BF16, tag="xt", name=f"xt{b}")
            nc.sync.dma_start(
                out=t[:, 0:2, :], in_=x_d[b, 0:2].rearrange("ec p l -> p ec l")
            )
            nc.sync.dma_start(
                out=t[:, 2:4, :], in_=x_d[b, 2:4].rearrange("ec p l -> p ec l")
            )
            xt[b] = t

        def mm_chunk(b, ci):
            c0, cw, _ = CCH[ci]
            tiles = []
            for lt in range(NLT):
                pv = ppool.tile([P, LTW], FP32, tag="pv", name=f"pv{ci}_{b}_{lt}")
                for ec in range(EC):
                    nc.tensor.matmul(
                        pv[:cw, :],
                        lhsT=w_sb[:, ec, c0 : c0 + cw],
                        rhs=xt[b][:, ec, lt * LTW : (lt + 1) * LTW],
                        start=(ec == 0),
                        stop=(ec == EC - 1),
                    )
                tiles.append(pv)
            if ci == 1:
                pv1[b] = tiles
            else:
                pv0[b] = tiles

        def score_prep(b):
            # u rows -> usb (zero-padded by 2 on both ends), then one diagonal
            # DMA (on SP, ahead of x prefetches) builds all 5 shifted rows.
            usb = upool.tile([WIN, L + 4], BF16, tag="usb", name=f"usb{b}")
            nc.gpsimd.memset(usb[:, 0:2], 0.0)
            nc.gpsimd.memset(usb[:, L + 2 : L + 4], 0.0)
            for lt in range(NLT):
                nc.scalar.copy(
                    out=usb[:, 2 + lt * LTW : 2 + (lt + 1) * LTW],
                    in_=pv1[b][lt][UOFF : UOFF + WIN, :],
                )
            ua = uapool.tile([WIN, L], BF16, tag="uali", name=f"uali{b}")
            src = usb[:, 0:L].copy()
            d = src.ap
            d[0] = (d[0][0] + 1, WIN)  # diagonal: row w starts w elements later
            nc.sync.dma_start(out=ua[:], in_=src)
            uali[b] = ua

        def score_mm(b):
            # broadcast-sum the 5 aligned rows to all partitions, sigmoid
            s = spool.tile([P, L], FP32, tag="ssb", name=f"ssb{b}")
            for lt in range(NLT):
                ps = ppool.tile([P, LTW], FP32, tag="pv", name=f"ps{b}_{lt}")
                nc.tensor.matmul(
                    ps[:],
                    lhsT=ones_sb[:],
                    rhs=uali[b][:, lt * LTW : (lt + 1) * LTW],
                    start=True,
                    stop=True,
                )
                nc.scalar.activation(
                    out=s[:, lt * LTW : (lt + 1) * LTW],
                    in_=ps[:],
                    func=AF.Sigmoid,
                    bias=batt_sb[:],
                )
            ssb[b] = s

        def gate_reduce(b, ci):
            # gate-multiply on DVE (reads psum) then max over l on DVE
            _, _, cwo = CCH[ci]
            tiles = pv1[b] if ci == 1 else pv0[b]
            g = gpool.tile([P, L], FP32, tag="gout", name=f"g{b}_{ci}")
            for lt in range(NLT):
                nc.vector.tensor_mul(
                    out=g[:cwo, lt * LTW : (lt + 1) * LTW],
                    in0=tiles[lt][:cwo, :],
                    in1=ssb[b][:cwo, lt * LTW : (lt + 1) * LTW],
                )
            nc.vector.reduce_max(
                oacc[ci][:cwo, b : b + 1],
                g[:cwo, :],
                axis=mybir.AxisListType.X,
            )
            if ci == 1:
                pv1[b] = None
            else:
                pv0[b] = None

        # ---- software-pipelined main loop ----
        load_x(0)
        if BS > 1:
            load_x(1)
        # cold constants + activation-table preloads behind x0/x1
        ones_sb = cpool.tile([WIN, P], BF16, tag="ones")
        nc.sync.dma_start(out=ones_sb[:], in_=ones_d)
        batt_sb = cpool.tile([P, 1], FP32, tag="batt")
        nc.sync.dma_start(out=batt_sb[:], in_=batt_d)
        bcnn_sb = []
        for ci, (c0, cw, cwo) in enumerate(CCH):
            t = cpool.tile([cwo, 1], FP32, tag=f"bcnn{ci}")
            nc.sync.dma_start(out=t[:], in_=bcnn_d[c0 : c0 + cwo, :])
            bcnn_sb.append(t)
        warm = cpool.tile([1, 2], FP32, tag="warm")
        nc.scalar.activation(out=warm[:, 0:1], in_=batt_sb[0:1, :], func=AF.Sigmoid)
        nc.scalar.activation(out=warm[:, 1:2], in_=batt_sb[0:1, :], func=AF.Tanh)

        for b in range(BS):
            mm_chunk(b, 1)       # PE: c-chunk 1 (contains u rows)
            score_prep(b)        # Act: u copy; SP: diagonal shift DMA
            if b + 2 < BS:
                load_x(b + 2)    # SP: behind this batch's diag DMA
            if b >= 1:
                score_mm(b - 1)      # PE: ones matmul; Act: sigmoid
                gate_reduce(b - 1, 1)  # DVE: gate+max, frees pv1(b-1)
                mm_chunk(b - 1, 0)   # PE: c-chunk 0
                gate_reduce(b - 1, 0)  # DVE: gate+max, frees pv0(b-1)
        b = BS - 1
        score_mm(b)
        gate_reduce(b, 1)
        mm_chunk(b, 0)
        gate_reduce(b, 0)

        # ---- tanh(max + b_cnn) and store ----
        for ci, (c0, cw, cwo) in enumerate(CCH):
            of = gpool.tile([P, BS], FP32, tag=f"of{ci}")
            nc.scalar.activation(
                out=of[:cwo, :], in_=oacc[ci][:cwo, :], func=AF.Tanh,
                bias=bcnn_sb[ci][:],
            )
            nc.sync.dma_start(out=out_d[c0 : c0 + cwo, :], in_=of[:cwo, :])


@functools.lru_cache(maxsize=1)
def _build():
    nc = bacc.Bacc(
        "TRN2",
        target_bir_lowering=False,
        debug=False,
        enable_asserts=False,
        num_devices=NCORES,
    )
    x_d = nc.dram_tensor("xT", [BS, EC, P, L], BF16, kind="ExternalInput").ap()
    w_d = nc.dram_tensor("waugT", [EC, P, CAUG], BF16, kind="ExternalInput").ap()
    ones_d = nc.dram_tensor("ones5", [WIN, P], BF16, kind="ExternalInput").ap()
    batt_d = nc.dram_tensor("b_att_b", [P, 1], FP32, kind="ExternalInput").ap()
    bcnn_d = nc.dram_tensor("b_cnn_c", [C, 1], FP32, kind="ExternalInput").ap()
    out_d = nc.dram_tensor("out", [C, BS], FP32, kind="ExternalOutput").ap()
    with tile.TileContext(nc) as tc:
        _body(nc, tc, x_d, w_d, ones_d, batt_d, bcnn_d, out_d)
    nc.compile()
    return nc


def _prep_in_maps(x, W_att, b_att, W_cnn, b_cnn):
    pad = np.zeros((CAUG - C - WIN, E), dtype=np.float32)
    waug = np.concatenate([W_cnn, pad, W_att], axis=0)     # [229, 512]
    waugT = np.ascontiguousarray(waug.T)                   # [512, 229]
    waugT = waugT.reshape(EC, P, CAUG).astype(ml_dtypes.bfloat16)
    ones5 = np.ones((WIN, P), dtype=ml_dtypes.bfloat16)
    batt = np.full((P, 1), np.float32(b_att[0]), dtype=np.float32)
    bcnn = np.asarray(b_cnn, dtype=np.float32).reshape(C, 1)
    # host-side cast + transpose: [B, L, E] -> bf16 [B, EC, P, L]
    xT = np.ascontiguousarray(
        np.asarray(x).astype(ml_dtypes.bfloat16).transpose(0, 2, 1)
    ).reshape(B, EC, P, L)
    in_maps = []
    for c in range(NCORES):
        in_maps.append(
            {
                "xT": xT[c * BS : (c + 1) * BS],
                "waugT": waugT,
                "ones5": ones5,
                "b_att_b": batt,
                "b_cnn_c": bcnn,
            }
        )
    return in_maps


def run(x, W_att, b_att, W_cnn, b_cnn, trace=False):
    nc = _build()
    in_maps = _prep_in_maps(x, W_att, b_att, W_cnn, b_cnn)
    res = run_bass_kernel_spmd(nc, in_maps, core_ids=list(range(NCORES)), trace=trace)
    outs = [r["out"] for r in res.results]  # each [C, BS]
    out = np.concatenate([o.T for o in outs], axis=0)  # [B, C]
    return out[:, :, None, None].astype(np.float32), res


def kernel(x, W_att, b_att, W_cnn, b_cnn):
    out, _ = run(x, W_att, b_att, W_cnn, b_cnn, trace=False)
    return out


# revision 18
# speedup vs baseline: 1.2489x; 1.1135x over previous
"""Trainium2 Bass kernel for LocalAttention: sliding-window attention gate +
per-position linear + tanh + global maxpool.

out[b,c] = tanh(max_l( sigmoid(conv1d(x, W_att) + b_att)[l] * (W_cnn @ x[b].T)[c,l] ) + b_cnn[c])

Sharding: data-parallel over batch B=64 across 8 cores (8 batches/core).

Key design points:
- x is cast to bf16 AND pre-transposed to [EC, 128, L] on the host, so the
  device does no cast and no on-chip transpose.
- W_cnn (200 rows) and W_att (5 rows) are packed into one 229-row augmented
  weight so the sliding-window score channels ride along in the main matmul.
- The 5 shifted score rows are realized with a single "diagonal" DMA whose
  partition stride also advances one element (stride = pitch+1); it is issued
  on the SP queue ahead of the x prefetches so it never queues behind them.
- One unified single-bank PSUM pool (8 bufs) lets c-chunk-1 tiles survive
  ~2 batches, giving the score chain (u-copy -> diag DMA -> ones matmul ->
  sigmoid) enough slack to stay off the critical path.
- Activation tables (Sigmoid/Tanh) are preloaded in the prologue.
"""

import functools
import sys

import ml_dtypes
import numpy as np

sys.path.insert(0, "/opt/trn_rl_repo")

import concourse.bacc as bacc
import concourse.tile as tile
from concourse import mybir
from concourse.bass_utils import run_bass_kernel_spmd

B, L, E, WIN, C = 64, 1024, 512, 5, 200
NCORES = 8
BS = B // NCORES  # batches per core
P = 128
EC = E // P       # 4 e-chunks (contraction over E in 128-slices)
NLT = 2           # L-tiles for matmul free dim
LTW = L // NLT    # 512
# augmented output channels: 200 cnn + zero-pad to 32-align + 5 att rows.
# u rows must start at a 32-aligned partition for compute-engine PSUM reads.
UOFF = 96         # local partition offset of the W_att rows inside c-chunk 1
CAUG = P + UOFF + WIN  # 229
# c-chunks of the augmented output: (start, width, valid_out_width)
CCH = [(0, P, P), (P, UOFF + WIN, C - P)]

FP32 = mybir.dt.float32
BF16 = mybir.dt.bfloat16
AF = mybir.ActivationFunctionType
ALU = mybir.AluOpType


def _body(nc, tc, x_d, w_d, ones_d, batt_d, bcnn_d, out_d):
    with (
        tc.tile_pool(name="const", bufs=1) as cpool,
        tc.tile_pool(name="xt", bufs=4) as xtpool,
        tc.tile_pool(name="u", bufs=2) as upool,
        tc.tile_pool(name="ua", bufs=2) as uapool,
        tc.tile_pool(name="s", bufs=2) as spool,
        tc.tile_pool(name="g", bufs=2) as gpool,
        tc.tile_pool(name="v", bufs=2) as vpool,
        tc.tile_pool(name="oacc", bufs=1) as opool,
        tc.tile_pool(name="pp", bufs=8, space="PSUM") as ppool,
    ):
        # ---- weights first so the first matmul's inputs land ASAP ----
        w_sb = cpool.tile([P, EC, CAUG], BF16, tag="w")
        nc.sync.dma_start(out=w_sb[:], in_=w_d.rearrange("ec p c -> p ec c"))

        oacc = [
            opool.tile([P, BS], FP32, tag=f"oacc{ci}", name=f"oacc{ci}")
            for ci in range(2)
        ]

        # ---- per-batch state kept across pipeline stages ----
        xt = [None] * BS     # SBUF x tiles [P, EC, L]
        pv1 = [None] * BS    # psum tiles for c-chunk 1 (with u rows), per lt
        pv0 = [None] * BS    # psum tiles for c-chunk 0, per lt
        uali = [None] * BS   # aligned u rows [WIN, L]
        ssb = [None] * BS    # sigmoid scores [P, L] fp32

        def load_x(b):
            t = xtpool.tile([P, EC, L], BF16, tag="xt", name=f"xt{b}")
            nc.sync.dma_start(
                out=t[:, 0:2, :], in_=x_d[b, 0:2].rearrange("ec p l -> p ec l")
            )
            nc.sync.dma_start(
                out=t[:, 2:4, :], in_=x_d[b, 2:4].rearrange("ec p l -> p ec l")
            )
            xt[b] = t

        def mm_chunk(b, ci):
            c0, cw, _ = CCH[ci]
            tiles = []
            for lt in range(NLT):
                pv = ppool.tile([P, LTW], FP32, tag="pv", name=f"pv{ci}_{b}_{lt}")
                for ec in range(EC):
                    nc.tensor.matmul(
                        pv[:cw, :],
                        lhsT=w_sb[:, ec, c0 : c0 + cw],
                        rhs=xt[b][:, ec, lt * LTW : (lt + 1) * LTW],
                        start=(ec == 0),
                        stop=(ec == EC - 1),
                    )
                tiles.append(pv)
            if ci == 1:
                pv1[b] = tiles
            else:
                pv0[b] = tiles

        def score_prep(b):
            # u rows -> usb (zero-padded by 2 on both ends), then one diagonal
            # DMA (on SP, ahead of x prefetches) builds all 5 shifted rows.
            usb = upool.tile([WIN, L + 4], BF16, tag="usb", name=f"usb{b}")
            nc.gpsimd.memset(usb[:, 0:2], 0.0)
            nc.gpsimd.memset(usb[:, L + 2 : L + 4], 0.0)
            for lt in range(NLT):
                nc.scalar.copy(
                    out=usb[:, 2 + lt * LTW : 2 + (lt + 1) * LTW],
                    in_=pv1[b][lt][UOFF : UOFF + WIN, :],
                )
            ua = uapool.tile([WIN, L], BF16, tag="uali", name=f"uali{b}")
            src = usb[:, 0:L].copy()
            d = src.ap
            d[0] = (d[0][0] + 1, WIN)  # diagonal: row w starts w elements later
            nc.sync.dma_start(out=ua[:], in_=src)
            uali[b] = ua

        def score_mm(b):
            # broadcast-sum the 5 aligned rows to all partitions, sigmoid
            s = spool.tile([P, L], FP32, tag="ssb", name=f"ssb{b}")
            for lt in range(NLT):
                ps = ppool.tile([P, LTW], FP32, tag="pv", name=f"ps{b}_{lt}")
                nc.tensor.matmul(
                    ps[:],
                    lhsT=ones_sb[:],
                    rhs=uali[b][:, lt * LTW : (lt + 1) * LTW],
                    start=True,
                    stop=True,
                )
                nc.scalar.activation(
                    out=s[:, lt * LTW : (lt + 1) * LTW],
                    in_=ps[:],
                    func=AF.Sigmoid,
                    bias=batt_sb[:],
                )
            ssb[b] = s

        def gate_reduce1(b):
            # c-chunk 1: gate-multiply on DVE (reads psum), max on DVE
            _, _, cwo = CCH[1]
            g = gpool.tile([P, L], FP32, tag="gout1", name=f"g{b}_1")
            for lt in range(NLT):
                nc.vector.tensor_mul(
                    out=g[:cwo, lt * LTW : (lt + 1) * LTW],
                    in0=pv1[b][lt][:cwo, :],
                    in1=ssb[b][:cwo, lt * LTW : (lt + 1) * LTW],
                )
            nc.vector.reduce_max(
                oacc[1][:cwo, b : b + 1],
                g[:cwo, :],
                axis=mybir.AxisListType.X,
            )
            pv1[b] = None

        def gate_reduce0(b):
            # c-chunk 0: psum -> SBUF on Act, gate-multiply on Pool, max on DVE
            _, _, cwo = CCH[0]
            v = vpool.tile([P, L], FP32, tag="vc", name=f"v{b}")
            for lt in range(NLT):
                nc.scalar.copy(
                    out=v[:cwo, lt * LTW : (lt + 1) * LTW],
                    in_=pv0[b][lt][:cwo, :],
                )
            g = gpool.tile([P, L], FP32, tag="gout0", name=f"g{b}_0")
            nc.gpsimd.tensor_mul(out=g[:cwo, :], in0=v[:cwo, :], in1=ssb[b][:cwo, :])
            nc.vector.reduce_max(
                oacc[0][:cwo, b : b + 1],
                g[:cwo, :],
                axis=mybir.AxisListType.X,
            )
            pv0[b] = None

        # ---- software-pipelined main loop ----
        load_x(0)
        if BS > 1:
            load_x(1)
        # cold constants + activation-table preloads behind x0/x1
        ones_sb = cpool.tile([WIN, P], BF16, tag="ones")
        nc.sync.dma_start(out=ones_sb[:], in_=ones_d)
        batt_sb = cpool.tile([P, 1], FP32, tag="batt")
        nc.sync.dma_start(out=batt_sb[:], in_=batt_d)
        bcnn_sb = []
        for ci, (c0, cw, cwo) in enumerate(CCH):
            t = cpool.tile([cwo, 1], FP32, tag=f"bcnn{ci}")
            nc.sync.dma_start(out=t[:], in_=bcnn_d[c0 : c0 + cwo, :])
            bcnn_sb.append(t)
        warm = cpool.tile([1, 2], FP32, tag="warm")
        nc.scalar.activation(out=warm[:, 0:1], in_=batt_sb[0:1, :], func=AF.Sigmoid)
        nc.scalar.activation(out=warm[:, 1:2], in_=batt_sb[0:1, :], func=AF.Tanh)

        for b in range(BS):
            mm_chunk(b, 1)       # PE: c-chunk 1 (contains u rows)
            score_prep(b)        # Act: u copy; SP: diagonal shift DMA
            if b + 2 < BS:
                load_x(b + 2)    # SP: behind this batch's diag DMA
            if b >= 1:
                score_mm(b - 1)      # PE: ones matmul; Act: sigmoid
                gate_reduce1(b - 1)  # DVE: gate+max, frees pv1(b-1)
                mm_chunk(b - 1, 0)   # PE: c-chunk 0
                gate_reduce0(b - 1)  # Act copy + Pool mul + DVE max
        b = BS - 1
        score_mm(b)
        gate_reduce1(b)
        mm_chunk(b, 0)
        gate_reduce0(b)

        # ---- tanh(max + b_cnn) and store ----
        for ci, (c0, cw, cwo) in enumerate(CCH):
            of = gpool.tile([P, BS], FP32, tag=f"of{ci}")
            nc.scalar.activation(
                out=of[:cwo, :], in_=oacc[ci][:cwo, :], func=AF.Tanh,
                bias=bcnn_sb[ci][:],
            )
            nc.sync.dma_start(out=out_d[c0 : c0 + cwo, :], in_=of[:cwo, :])


@functools.lru_cache(maxsize=1)
def _build():
    nc = bacc.Bacc(
        "TRN2",
        target_bir_lowering=False,
        debug=False,
        enable_asserts=False,
        num_devices=NCORES,
    )
    x_d = nc.dram_tensor("xT", [BS, EC, P, L], BF16, kind="ExternalInput").ap()
    w_d = nc.dram_tensor("waugT", [EC, P, CAUG], BF16, kind="ExternalInput").ap()
    ones_d = nc.dram_tensor("ones5", [WIN, P], BF16, kind="ExternalInput").ap()
    batt_d = nc.dram_tensor("b_att_b", [P, 1], FP32, kind="ExternalInput").ap()
    bcnn_d = nc.dram_tensor("b_cnn_c", [C, 1], FP32, kind="ExternalInput").ap()
    out_d = nc.dram_tensor("out", [C, BS], FP32, kind="ExternalOutput").ap()
    with tile.TileContext(nc) as tc:
        _body(nc, tc, x_d, w_d, ones_d, batt_d, bcnn_d, out_d)
    nc.compile()
    return nc


def _prep_in_maps(x, W_att, b_att, W_cnn, b_cnn):
    pad = np.zeros((CAUG - C - WIN, E), dtype=np.float32)
    waug = np.concatenate([W_cnn, pad, W_att], axis=0)     # [229, 512]
    waugT = np.ascontiguousarray(waug.T)                   # [512, 229]
    waugT = waugT.reshape(EC, P, CAUG).astype(ml_dtypes.bfloat16)
    ones5 = np.ones((WIN, P), dtype=ml_dtypes.bfloat16)
    batt = np.full((P, 1), np.float32(b_att[0]), dtype=np.float32)
    bcnn = np.asarray(b_cnn, dtype=np.float32).reshape(C, 1)
    # host-side cast + transpose: [B, L, E] -> bf16 [B, EC, P, L]
    xT = np.ascontiguousarray(
        np.asarray(x).astype(ml_dtypes.bfloat16).transpose(0, 2, 1)
    ).reshape(B, EC, P, L)
    in_maps = []
    for c in range(NCORES):
        in_maps.append(
            {
                "xT": xT[c * BS : (c + 1) * BS],
                "waugT": waugT,
                "ones5": ones5,
                "b_att_b": batt,
                "b_cnn_c": bcnn,
            }
        )
    return in_maps


def run(x, W_att, b_att, W_cnn, b_cnn, trace=False):
    nc = _build()
    in_maps = _prep_in_maps(x, W_att, b_att, W_cnn, b_cnn)
    res = run_bass_kernel_spmd(nc, in_maps, core_ids=list(range(NCORES)), trace=trace)
    outs = [r["out"] for r in res.results]  # each [C, BS]
    out = np.concatenate([o.T for o in outs], axis=0)  # [B, C]
    return out[:, :, None, None].astype(np.float32), res


def kernel(x, W_att, b_att, W_cnn, b_cnn):
    out, _ = run(x, W_att, b_att, W_cnn, b_cnn, trace=False)
    return out


# revision 21
# speedup vs baseline: 1.3008x; 1.0415x over previous
"""Trainium2 Bass kernel for LocalAttention: sliding-window attention gate +
per-position linear + tanh + global maxpool.

out[b,c] = tanh(max_l( sigmoid(conv1d(x, W_att) + b_att)[l] * (W_cnn @ x[b].T)[c,l] ) + b_cnn[c])

Sharding: data-parallel over batch B=64 across 8 cores (8 batches/core).

Key design points:
- x is cast to bf16 AND pre-transposed to [EC, 128, L] on the host, so the
  device does no cast and no on-chip transpose.
- W_cnn (200 rows) and W_att (5 rows) are packed into one 229-row augmented
  weight so the sliding-window score channels ride along in the main matmul.
- The 5 shifted score rows are realized with a single "diagonal" DMA whose
  partition stride also advances one element (stride = pitch+1); it is issued
  on the SP queue ahead of the x prefetches so it never queues behind them.
- One unified single-bank PSUM pool (8 bufs) lets c-chunk-1 tiles survive
  ~2 batches, giving the score chain (u-copy -> diag DMA -> ones matmul ->
  sigmoid) enough slack to stay off the critical path.
- Activation tables (Sigmoid/Tanh) are preloaded in the prologue.
"""

import functools
import sys

import ml_dtypes
import numpy as np

sys.path.insert(0, "/opt/trn_rl_repo")

import concourse.bacc as bacc
import concourse.tile as tile
from concourse import mybir
from concourse.bass_utils import run_bass_kernel_spmd

B, L, E, WIN, C = 64, 1024, 512, 5, 200
NCORES = 8
BS = B // NCORES  # batches per core
P = 128
EC = E // P       # 4 e-chunks (contraction over E in 128-slices)
NLT = 2           # L-tiles for matmul free dim
LTW = L // NLT    # 512
# augmented output channels: 200 cnn + zero-pad to 32-align + 5 att rows.
# u rows must start at a 32-aligned partition for compute-engine PSUM reads.
UOFF = 96         # local partition offset of the W_att rows inside c-chunk 1
CAUG = P + UOFF + WIN  # 229
# c-chunks of the augmented output: (start, width, valid_out_width)
CCH = [(0, P, P), (P, UOFF + WIN, C - P)]

FP32 = mybir.dt.float32
BF16 = mybir.dt.bfloat16
AF = mybir.ActivationFunctionType
ALU = mybir.AluOpType


def _body(nc, tc, x_d, w_d, ones_d, batt_d, bcnn_d, out_d):
    with (
        tc.tile_pool(name="const", bufs=1) as cpool,
        tc.tile_pool(name="xt", bufs=4) as xtpool,
        tc.tile_pool(name="u", bufs=2) as upool,
        tc.tile_pool(name="ua", bufs=2) as uapool,
        tc.tile_pool(name="s", bufs=2) as spool,
        tc.tile_pool(name="g", bufs=2) as gpool,
        tc.tile_pool(name="v", bufs=2) as vpool,
        tc.tile_pool(name="oacc", bufs=1) as opool,
        tc.tile_pool(name="pp", bufs=8, space="PSUM") as ppool,
    ):
        # ---- weights first so the first matmul's inputs land ASAP; the
        # ci1 slice (101 cols, incl. u rows) goes ahead of the ci0 slice so
        # batch 0's first chunk can start earliest
        w_sb = cpool.tile([P, EC, CAUG], BF16, tag="w")
        nc.sync.dma_start(
            out=w_sb[:, :, P:CAUG],
            in_=w_d[:, :, P:CAUG].rearrange("ec p c -> p ec c"),
        )

        # warm-up: keep the PE busy during the initial DMA wait so the
        # p-state ramp reaches full speed before the first real matmul
        warm_sb = cpool.tile([P, 256], BF16, tag="warmmm")
        nc.gpsimd.memset(warm_sb[:], 0.0)
        warm_ps = ppool.tile([P, 256], FP32, tag="pv", name="warm_ps")
        for _ in range(22):
            nc.tensor.matmul(
                warm_ps[:], lhsT=warm_sb[:, 0:P], rhs=warm_sb[:],
                start=True, stop=True,
            )

        oacc = [
            opool.tile([P, BS], FP32, tag=f"oacc{ci}", name=f"oacc{ci}")
            for ci in range(2)
        ]

        # ---- per-batch state kept across pipeline stages ----
        xt = [None] * BS     # SBUF x tiles [P, EC, L]
        pv1 = [None] * BS    # psum tiles for c-chunk 1 (with u rows), per lt
        pv0 = [None] * BS    # psum tiles for c-chunk 0, per lt
        uali = [None] * BS   # aligned u rows [WIN, L]
        ssb = [None] * BS    # sigmoid scores [P, L] fp32

        def load_x(b, quarters=False):
            t = xtpool.tile([P, EC, L], BF16, tag="xt", name=f"xt{b}")
            step = 1 if quarters else 2
            for e0 in range(0, EC, step):
                nc.sync.dma_start(
                    out=t[:, e0 : e0 + step, :],
                    in_=x_d[b, e0 : e0 + step].rearrange("ec p l -> p ec l"),
                )
            xt[b] = t

        def mm_chunk(b, ci):
            c0, cw, _ = CCH[ci]
            tiles = []
            for lt in range(NLT):
                pv = ppool.tile([P, LTW], FP32, tag="pv", name=f"pv{ci}_{b}_{lt}")
                for ec in range(EC):
                    nc.tensor.matmul(
                        pv[:cw, :],
                        lhsT=w_sb[:, ec, c0 : c0 + cw],
                        rhs=xt[b][:, ec, lt * LTW : (lt + 1) * LTW],
                        start=(ec == 0),
                        stop=(ec == EC - 1),
                    )
                tiles.append(pv)
            if ci == 1:
                pv1[b] = tiles
            else:
                pv0[b] = tiles

        def score_prep(b):
            # u rows -> usb (zero-padded by 2 on both ends), then one diagonal
            # DMA (on SP, ahead of x prefetches) builds all 5 shifted rows.
            usb = upool.tile([WIN, L + 4], BF16, tag="usb", name=f"usb{b}")
            nc.gpsimd.memset(usb[:, 0:2], 0.0)
            nc.gpsimd.memset(usb[:, L + 2 : L + 4], 0.0)
            for lt in range(NLT):
                nc.scalar.copy(
                    out=usb[:, 2 + lt * LTW : 2 + (lt + 1) * LTW],
                    in_=pv1[b][lt][UOFF : UOFF + WIN, :],
                )
            ua = uapool.tile([WIN, L], BF16, tag="uali", name=f"uali{b}")
            src = usb[:, 0:L].copy()
            d = src.ap
            d[0] = (d[0][0] + 1, WIN)  # diagonal: row w starts w elements later
            nc.sync.dma_start(out=ua[:], in_=src)
            uali[b] = ua

        def score_mm(b):
            # broadcast-sum the 5 aligned rows to all partitions, sigmoid
            s = spool.tile([P, L], FP32, tag="ssb", name=f"ssb{b}")
            for lt in range(NLT):
                ps = ppool.tile([P, LTW], FP32, tag="pv", name=f"ps{b}_{lt}")
                nc.tensor.matmul(
                    ps[:],
                    lhsT=ones_sb[:],
                    rhs=uali[b][:, lt * LTW : (lt + 1) * LTW],
                    start=True,
                    stop=True,
                )
                nc.scalar.activation(
                    out=s[:, lt * LTW : (lt + 1) * LTW],
                    in_=ps[:],
                    func=AF.Sigmoid,
                    bias=batt_sb[:],
                )
            ssb[b] = s

        def gate_reduce1(b):
            # c-chunk 1: gate-multiply on DVE (reads psum), max on DVE
            _, _, cwo = CCH[1]
            g = gpool.tile([P, L], FP32, tag="gout1", name=f"g{b}_1")
            for lt in range(NLT):
                nc.vector.tensor_mul(
                    out=g[:cwo, lt * LTW : (lt + 1) * LTW],
                    in0=pv1[b][lt][:cwo, :],
                    in1=ssb[b][:cwo, lt * LTW : (lt + 1) * LTW],
                )
            nc.vector.reduce_max(
                oacc[1][:cwo, b : b + 1],
                g[:cwo, :],
                axis=mybir.AxisListType.X,
            )
            pv1[b] = None

        def gate_reduce0(b):
            # c-chunk 0: psum -> SBUF on Act, gate-multiply on Pool, max on DVE
            _, _, cwo = CCH[0]
            v = vpool.tile([P, L], FP32, tag="vc", name=f"v{b}")
            for lt in range(NLT):
                nc.scalar.copy(
                    out=v[:cwo, lt * LTW : (lt + 1) * LTW],
                    in_=pv0[b][lt][:cwo, :],
                )
            g = gpool.tile([P, L], FP32, tag="gout0", name=f"g{b}_0")
            nc.gpsimd.tensor_mul(out=g[:cwo, :], in0=v[:cwo, :], in1=ssb[b][:cwo, :])
            nc.vector.reduce_max(
                oacc[0][:cwo, b : b + 1],
                g[:cwo, :],
                axis=mybir.AxisListType.X,
            )
            pv0[b] = None

        # ---- software-pipelined main loop ----
        load_x(0, quarters=True)
        # ci0 weight slice after x0
        nc.sync.dma_start(
            out=w_sb[:, :, 0:P], in_=w_d[:, :, 0:P].rearrange("ec p c -> p ec c")
        )
        if BS > 1:
            load_x(1)
        # cold constants + activation-table preloads behind x0/x1
        ones_sb = cpool.tile([WIN, P], BF16, tag="ones")
        nc.sync.dma_start(out=ones_sb[:], in_=ones_d)
        batt_sb = cpool.tile([P, 1], FP32, tag="batt")
        nc.sync.dma_start(out=batt_sb[:], in_=batt_d)
        bcnn_sb = []
        for ci, (c0, cw, cwo) in enumerate(CCH):
            t = cpool.tile([cwo, 1], FP32, tag=f"bcnn{ci}")
            nc.sync.dma_start(out=t[:], in_=bcnn_d[c0 : c0 + cwo, :])
            bcnn_sb.append(t)
        warm = cpool.tile([1, 2], FP32, tag="warm")
        nc.scalar.activation(out=warm[:, 0:1], in_=batt_sb[0:1, :], func=AF.Sigmoid)
        nc.scalar.activation(out=warm[:, 1:2], in_=batt_sb[0:1, :], func=AF.Tanh)

        for b in range(BS):
            mm_chunk(b, 1)       # PE: c-chunk 1 (contains u rows)
            score_prep(b)        # Act: u copy; SP: diagonal shift DMA
            if b + 2 < BS:
                load_x(b + 2)    # SP: behind this batch's diag DMA
            if b >= 1:
                score_mm(b - 1)      # PE: ones matmul; Act: sigmoid
                gate_reduce1(b - 1)  # DVE: gate+max, frees pv1(b-1)
                mm_chunk(b - 1, 0)   # PE: c-chunk 0
                gate_reduce0(b - 1)  # Act copy + Pool mul + DVE max
        b = BS - 1
        score_mm(b)
        gate_reduce1(b)
        mm_chunk(b, 0)
        gate_reduce0(b)

        # ---- tanh(max + b_cnn) and store ----
        for ci, (c0, cw, cwo) in enumerate(CCH):
            of = gpool.tile([P, BS], FP32, tag=f"of{ci}")
            nc.scalar.activation(
                out=of[:cwo, :], in_=oacc[ci][:cwo, :], func=AF.Tanh,
                bias=bcnn_sb[ci][:],
            )
            nc.sync.dma_start(out=out_d[c0 : c0 + cwo, :], in_=of[:cwo, :])


@functools.lru_cache(maxsize=1)
def _build():
    nc = bacc.Bacc(
        "TRN2",
        target_bir_lowering=False,
        debug=False,
        enable_asserts=False,
        num_devices=NCORES,
    )
    x_d = nc.dram_tensor("xT", [BS, EC, P, L], BF16, kind="ExternalInput").ap()
    w_d = nc.dram_tensor("waugT", [EC, P, CAUG], BF16, kind="ExternalInput").ap()
    ones_d = nc.dram_tensor("ones5", [WIN, P], BF16, kind="ExternalInput").ap()
    batt_d = nc.dram_tensor("b_att_b", [P, 1], FP32, kind="ExternalInput").ap()
    bcnn_d = nc.dram_tensor("b_cnn_c", [C, 1], FP32, kind="ExternalInput").ap()
    out_d = nc.dram_tensor("out", [C, BS], FP32, kind="ExternalOutput").ap()
    with tile.TileContext(nc) as tc:
        _body(nc, tc, x_d, w_d, ones_d, batt_d, bcnn_d, out_d)
    nc.compile()
    return nc


def _prep_in_maps(x, W_att, b_att, W_cnn, b_cnn):
    pad = np.zeros((CAUG - C - WIN, E), dtype=np.float32)
    waug = np.concatenate([W_cnn, pad, W_att], axis=0)     # [229, 512]
    waugT = np.ascontiguousarray(waug.T)                   # [512, 229]
    waugT = waugT.reshape(EC, P, CAUG).astype(ml_dtypes.bfloat16)
    ones5 = np.ones((WIN, P), dtype=ml_dtypes.bfloat16)
    batt = np.full((P, 1), np.float32(b_att[0]), dtype=np.float32)
    bcnn = np.asarray(b_cnn, dtype=np.float32).reshape(C, 1)
    # host-side cast + transpose: [B, L, E] -> bf16 [B, EC, P, L]
    xT = np.ascontiguousarray(
        np.asarray(x).astype(ml_dtypes.bfloat16).transpose(0, 2, 1)
    ).reshape(B, EC, P, L)
    in_maps = []
    for c in range(NCORES):
        in_maps.append(
            {
                "xT": xT[c * BS : (c + 1) * BS],
                "waugT": waugT,
                "ones5": ones5,
                "b_att_b": batt,
                "b_cnn_c": bcnn,
            }
        )
    return in_maps


def run(x, W_att, b_att, W_cnn, b_cnn, trace=False):
    nc = _build()
    in_maps = _prep_in_maps(x, W_att, b_att, W_cnn, b_cnn)
    res = run_bass_kernel_spmd(nc, in_maps, core_ids=list(range(NCORES)), trace=trace)
    outs = [r["out"] for r in res.results]  # each [C, BS]
    out = np.concatenate([o.T for o in outs], axis=0)  # [B, C]
    return out[:, :, None, None].astype(np.float32), res


def kernel(x, W_att, b_att, W_cnn, b_cnn):
    out, _ = run(x, W_att, b_att, W_cnn, b_cnn, trace=False)
    return out
